# revision 1
# baseline (speedup 1.0000x reference)
"""Trainium2 kernel for nn_BalancedHamiltonLayer.

Math: out = einsum("btd,rde->bte", x, factors)/sqrt(rank) + bias.
The einsum contracts r as a plain sum, so sum_r (x @ F_r) == x @ (sum_r F_r):
one [16384,2048] @ [2048,2048] GEMM instead of eight.

Distribution over 8 NeuronCores (single SPMD program):
  - x is row-sharded over b*t: core c owns rows [c*2048, (c+1)*2048).
  - factors are column-sharded for the reduction: core c reduces
    W_c = sum_r factors[r, :, c*256:(c+1)*256] on-device (DVE tree adds),
    then two AllGathers (one per 128-wide e-half of W_c) replicate the
    full W to every core. The e-halves are independent output columns,
    so the GEMM runs e-tile by e-tile: the first AllGather unblocks half
    the GEMM and the second hides completely under it.
  - GEMM per core: x^T stays fully SBUF-resident in fp32r; W e-tiles
    [128d x 128e] stream through SBUF as the stationary operand
    (out^T = W_tile.T @ x^T at full fp32r PE rate). The 1/sqrt(8) scale
    and bias land in the PSUM eviction (DVE tensor_scalar, bias is
    per-partition in the transposed layout).
  - Each core writes out^T [2048e, 2048m]; the host transposes back.

Host side only shards/lays out inputs (partition-major swizzle so every DMA
is contiguous per partition) and reassembles the per-core outputs.
"""

import math

import numpy as np

B, T, DIM, RANK = 4, 4096, 2048, 8
N_CORES = 8
MC = (B * T) // N_CORES        # 2048 rows per core
EC = DIM // N_CORES            # 256 output cols reduced per core
NT = DIM // 128                # 16 contraction tiles
NJ = MC // 256                 # 8 m-super-tiles per core
NET = 2 * N_CORES              # 16 e-tiles of 128 cols
SCALE = 1.0 / math.sqrt(RANK)

_CACHE = {}


def _build():
    import concourse.bacc as bacc
    import concourse.mybir as mybir
    import concourse.tile as tile

    f32 = mybir.dt.float32
    f32r = mybir.dt.float32r
    add = mybir.AluOpType.add
    mult = mybir.AluOpType.mult
    grp = [list(range(N_CORES))]

    nc = bacc.Bacc(
        "TRN2", target_bir_lowering=False, debug=False, num_devices=N_CORES
    )
    # [J, p, t, m]: x^T tiles, d = t*128+p, m_global = J*512+m
    xh = nc.dram_tensor("xh", [NJ // 2, 128, NT, 512], f32r, kind="ExternalInput").ap()
    # [r, eh, p, t, e]: this core's factor slice, d = t*128+p,
    # e_global = 256*core + 128*eh + e
    fh = nc.dram_tensor(
        "fh", [2, RANK // 2, 128, 2, NT, 128], f32, kind="ExternalInput"
    ).ap()
    # [p, et]: bias for e-tile et=(eh*8+r) at partition p:
    # bias_cols[p, et] = bias[r*256 + eh*128 + p]
    bias_cols = nc.dram_tensor("bias_cols", [128, NET], f32, kind="ExternalInput").ap()
    # transposed output: outT[e, m]
    outT = nc.dram_tensor("outT", [DIM, MC], f32, kind="ExternalOutput").ap()

    with tile.TileContext(nc) as tc:
        with (
            tc.tile_pool(name="const", bufs=1) as const_pool,
            tc.tile_pool(name="dram", bufs=1, space="DRAM") as dram_pool,
            tc.tile_pool(name="xa", bufs=1) as xa_pool,
        ):
            scope = nc.named_scope
            bias_sb = const_pool.tile([128, NET], f32)
            nc.sync.dma_start(bias_sb[:], bias_cols[:])

            wc_half = [
                dram_pool.tile([128, NT, 128], f32r, name=f"wc_half{i}")
                for i in range(2)
            ]
            w_half = [
                dram_pool.tile(
                    [N_CORES, 128, NT, 128], f32r,
                    addr_space="Shared", name=f"w_half{i}",
                )
                for i in range(2)
            ]

            # Phase 1: W_c = sum_r fh[r]. One 1 MB load per (rank, e-half),
            # all on the scalar HWDGE ring (sync ring only carries the wc
            # stores, so no head-of-line blocking). DVE tree adds per
            # (e-half, t-half), two leaf adds on GpSimd. Each e-half's
            # AllGather fires as soon as that half is stored.
            with (
                tc.tile_pool(name="red", bufs=6) as red_pool,
                tc.tile_pool(name="racc", bufs=1) as acc_pool,
            ):
                last_fload = None
                for eh in range(2):
                  with scope(f"reduce{eh}"):
                    pr = []
                    for rp in range(RANK // 2):
                        p_t = red_pool.tile([128, 2, NT, 128], f32, tag="fr")
                        eng = nc.scalar if rp % 2 == 0 else nc.sync
                        last_fload = eng.dma_start(p_t[:], fh[eh, rp])  # [p,q,t,e]
                        pr.append(p_t)
                    sA = acc_pool.tile([128, NT, 128], f32, tag="s0")
                    sB = acc_pool.tile([128, NT, 128], f32, tag="s1")
                    nc.vector.tensor_add(sA[:], pr[0][:, 0], pr[0][:, 1])
                    nc.vector.tensor_add(sB[:], pr[1][:, 0], pr[1][:, 1])
                    nc.vector.tensor_add(sA[:], sA[:], sB[:])
                    # sB freed; reused below for the gpsimd half
                    sC = acc_pool.tile([128, NT, 128], f32, tag="s2")
                    nc.vector.tensor_add(sB[:], pr[2][:, 0], pr[2][:, 1])
                    nc.vector.tensor_add(sC[:], pr[3][:, 0], pr[3][:, 1])
                    nc.vector.tensor_add(sB[:], sB[:], sC[:])
                    sfin = acc_pool.tile([128, NT, 128], f32r, tag="sf")
                    nc.vector.tensor_add(sfin[:], sA[:], sB[:])
                    nc.gpsimd.dma_start(wc_half[eh][:], sfin[:])
                    with scope(f"ag{eh}"):
                        nc.gpsimd.collective_compute(
                            "AllGather", mybir.AluOpType.bypass,
                            ins=[wc_half[eh].opt()],
                            outs=[w_half[eh].opt()],
                            replica_groups=grp,
                        )

            # x^T first half on the HWDGE rings: FIFO order guarantees the
            # factor loads drain first. xh is declared f32r (host feeds raw
            # fp32 bits; the PE's f32r path truncates identically).
            xa = xa_pool.tile([128, NT, 2, 512], f32r)
            with tc.tile_wait_until(0.025):
                for J in range(2):
                    eng = nc.sync if J % 2 == 0 else nc.scalar
                    eng.dma_start(xa[:, :, J, :], xh[J])

            # Phase 3: out^T[e,:] per 128-wide e-tile; W tile is the
            # stationary operand, resident x^T streams through the PE.
            with (
                tc.tile_pool(name="xb", bufs=1) as xb_pool,
                tc.tile_pool(name="wsb", bufs=4) as wpool,
                tc.tile_pool(name="osb", bufs=2) as opool,
                tc.tile_pool(name="ps", bufs=2, space="PSUM") as ppool,
            ):
                xb = xb_pool.tile([128, NT, 2, 512], f32r)
                with tc.tile_wait_until(0.045):
                    for J in range(2):
                        eng = nc.sync if J % 2 == 0 else nc.scalar
                        eng.dma_start(xb[:, :, J, :], xh[2 + J])
                for et in range(NET):
                    eh, r = et // N_CORES, et % N_CORES
                    wsb = wpool.tile([128, NT, 128], f32r, tag="wsb")
                    nc.gpsimd.dma_start(wsb[:], w_half[eh][r])
                    with scope(f"gemm{et}"):
                        ps = ppool.tile([128, 4, 512], f32, tag="ps")
                        for mc in range(4):
                            xsrc = xa if mc < 2 else xb
                            for t in range(NT):
                                nc.tensor.matmul(
                                    ps[:, mc, :],
                                    wsb[:, t, :],
                                    xsrc[:, t, mc % 2, :],
                                    start=(t == 0),
                                    stop=(t == NT - 1),
                                )
                        osb = opool.tile([128, MC], f32, tag="osb")
                        nc.vector.tensor_scalar(
                            osb[:], ps.rearrange("p a b -> p (a b)"),
                            SCALE, bias_sb[:, et : et + 1], mult, add,
                        )
                        e0 = r * EC + eh * 128
                        nc.scalar.dma_start(outT[e0 : e0 + 128, :], osb[:])

    nc.compile()
    return nc


def _get_nc():
    if "nc" not in _CACHE:
        _CACHE["nc"] = _build()
    return _CACHE["nc"]


def _shard(x, factors, bias):
    x_flat = np.ascontiguousarray(x, dtype=np.float32).reshape(B * T, DIM)
    factors = np.ascontiguousarray(factors, dtype=np.float32)
    bias = np.ascontiguousarray(bias, dtype=np.float32)
    # bias_cols[p, eh*8+r] = bias[r*256 + eh*128 + p]
    bias_cols = np.ascontiguousarray(
        bias.reshape(RANK, 2, 128).transpose(2, 1, 0).reshape(128, NET)
    )
    in_maps = []
    for c in range(N_CORES):
        xc = x_flat[c * MC : (c + 1) * MC, :]          # [m, d]
        # -> [J, p, t, m_local] with d = t*128+p, m = J*256+m_local
        xh = np.ascontiguousarray(
            xc.T.reshape(NT, 128, NJ // 2, 512).transpose(2, 1, 0, 3)
        )
        fc = factors[:, :, c * EC : (c + 1) * EC]       # [r, d, e]
        # -> [eh, rpair, p, q, t, e128] with r = 2*rpair + q, d = t*128 + p
        fhc = np.ascontiguousarray(
            fc.reshape(RANK // 2, 2, NT, 128, 2, 128).transpose(4, 0, 3, 1, 2, 5)
        )
        in_maps.append({"xh": xh, "fh": fhc, "bias_cols": bias_cols})
    return in_maps


def _run(in_maps, trace=False, trace_cores=None):
    from concourse.bass_utils import run_bass_kernel_spmd

    nc = _get_nc()
    return run_bass_kernel_spmd(
        nc, in_maps, list(range(N_CORES)), trace=trace, trace_cores=trace_cores
    )


def _assemble(res):
    out = np.empty((B * T, DIM), dtype=np.float32)
    for c in range(N_CORES):
        out[c * MC : (c + 1) * MC, :] = res.results[c]["outT"].T
    return out.reshape(B, T, DIM)


def kernel(x, factors, bias):
    res = _run(_shard(x, factors, bias), trace=False)
    return _assemble(res)



# revision 8
# speedup vs baseline: 1.3112x; 1.3112x over previous
"""Trainium2 kernel for nn_BalancedHamiltonLayer.

Math: out = einsum("btd,rde->bte", x, factors)/sqrt(rank) + bias.
The einsum contracts r as a plain sum, so sum_r (x @ F_r) == x @ (sum_r F_r):
one [16384,2048] @ [2048,2048] GEMM instead of eight.

Distribution over 8 NeuronCores (single SPMD program), DP2 x TP4:
  - core c = (dp, tp) with dp = c//4, tp = c%4 owns output rows
    [dp*8192, (dp+1)*8192) and output cols [tp*512, (tp+1)*512).
  - W reduction is split between the two cores sharing a tp column group:
    core (dp, tp) reduces cols [tp*512 + dp*256, +256) from its factor
    slice (bf16 strided-pair adds on DVE: 3 instructions per 128-col
    chunk instead of a 7-add tree), then one tiny pairwise AllGather
    (1 MB, groups [[0,4],[1,5],[2,6],[3,7]]) swaps the halves.
  - SPMD slot trick: every core computes W_peer = (slot0 + slot1) -
    W_local in fp32 (exact: W_local equals one slot bitwise, so the
    subtraction recovers the other slot's bf16 values exactly). This
    keeps the program identical across cores with no per-core indexing.
  - GEMM in bf16 (fp32 PSUM): phase A = first PRE m-tiles against the
    locally reduced cols (runs in the AllGather's shadow), phase B =
    same m-tiles against peer cols, phase C = remaining m-tiles against
    all four 128-col e-tiles. 1/sqrt(8) scale + bias fused into the
    PSUM eviction (scalar_tensor_tensor on DVE).
  - Each core writes out^T [512e, 8192m] fp32; the host transposes back.

Host side shards/lays out inputs (partition-major swizzle, bf16 cast) and
reassembles the per-core outputs.
"""

import math

import numpy as np

B, T, DIM, RANK = 4, 4096, 2048, 8
N_CORES = 8
DP, TP = 2, 4
MC = (B * T) // DP             # 8192 rows per core
ECO = DIM // TP                # 512 output cols per core
ECL = ECO // DP                # 256 cols reduced locally
NT = DIM // 128                # 16 contraction tiles
NMJ = MC // 512                # 16 m-supertiles per core
PRE = 5                        # m-tiles run split (local cols first)
SCALE = 1.0 / math.sqrt(RANK)

_CACHE = {}


def _build():
    import concourse.bacc as bacc
    import concourse.mybir as mybir
    import concourse.tile as tile

    f32 = mybir.dt.float32
    bf16 = mybir.dt.bfloat16
    add = mybir.AluOpType.add
    mult = mybir.AluOpType.mult
    pair_groups = [[c, c + 4] for c in range(4)]

    nc = bacc.Bacc(
        "TRN2", target_bir_lowering=False, debug=False, num_devices=N_CORES
    )
    # x^T tiles: d = t*128+p, m_global = dp*8192 + mj*512 + m
    xh = nc.dram_tensor("xh", [NMJ, 128, NT, 512], bf16, kind="ExternalInput").ap()
    # local factor cols: [ec, p, rp, q, t, e]; r = rp*2+q, d = t*128+p,
    # e_global = tp*512 + dp*256 + ec*128 + e
    fh = nc.dram_tensor(
        "fh", [2, 128, RANK // 2, 2, NT, 128], bf16, kind="ExternalInput"
    ).ap()
    # bias per (p, slot) replicated over m; slot order [L0, L1, P0, P1]
    bias_melt = nc.dram_tensor(
        "bias_melt", [128, 4, 512], f32, kind="ExternalInput"
    ).ap()
    # transposed output: rows = [L0 L1 P0 P1] col chunks, cols = m
    outT = nc.dram_tensor("outT", [ECO, MC], f32, kind="ExternalOutput").ap()

    with tile.TileContext(nc) as tc:
        with (
            tc.tile_pool(name="const", bufs=1) as const_pool,
            tc.tile_pool(name="dram", bufs=1, space="DRAM") as dram_pool,
            tc.tile_pool(name="wsb", bufs=1) as w_pool,
            tc.tile_pool(name="xa", bufs=5) as x_pool,
        ):
            scope = nc.named_scope
            # x0 ahead of bias on scalar: x0 gates the first matmul.
            xsb = [x_pool.tile([128, NT, 512], bf16, tag="x", name=f"x{i}")
                   for i in range(NMJ)]
            nc.scalar.dma_start(xsb[0][:], xh[0])
            bias_sb = const_pool.tile([128, 4, 512], f32)
            nc.scalar.dma_start(bias_sb[:], bias_melt[:])

            wc = dram_pool.tile([128, 2, NT, 128], bf16, name="wc")
            w_pair = dram_pool.tile([2, 128, 2, NT, 128], bf16, name="w_pair")

            # Early x tiles: x2..x5 on gpsimd; x1 on sync behind the two
            # factor loads. Late tiles are prefetched inside the GEMM
            # loops (pool backpressure would otherwise head-of-line block
            # the rings behind unfreed slots).
            for mj in range(2, 5):
                nc.gpsimd.dma_start(xsb[mj][:], xh[mj])

            # Phase R: local W chunks. One 1 MB factor load per e-chunk on
            # sync; 3 strided-pair adds per chunk on DVE (bf16 2x mode).
            wl = w_pool.tile([128, 2, NT, 128], bf16)
            with tc.tile_pool(name="red", bufs=1) as red_pool:
                fsb = []
                for ec in range(2):
                    ft = red_pool.tile([128, RANK // 2, 2, NT, 128], bf16,
                                       tag="fr")
                    nc.sync.dma_start(ft[:], fh[ec])
                    fsb.append(ft)
                nc.sync.dma_start(xsb[1][:], xh[1])
                for ec in range(2):
                    with scope(f"reduce{ec}"):
                        s1 = red_pool.tile([128, RANK // 2, NT, 128], bf16,
                                           tag="s1")
                        nc.vector.tensor_add(s1[:], fsb[ec][:, :, 0],
                                             fsb[ec][:, :, 1])
                        s2 = red_pool.tile([128, 2, NT, 128], bf16, tag="s2")
                        nc.vector.tensor_add(s2[:], s1[:, 0:2], s1[:, 2:4])
                        nc.vector.tensor_add(wl[:, ec], s2[:, 0], s2[:, 1])
                        nc.sync.dma_start(wc[:, ec], wl[:, ec])

            # Pairwise AllGather of the two local chunks (1 MB in, 2 MB out).
            with scope("ag"):
                nc.gpsimd.collective_compute(
                    "AllGather", mybir.AluOpType.bypass,
                    ins=[wc.opt()],
                    outs=[w_pair.opt()],
                    replica_groups=pair_groups,
                )

            # Peer W: slot0 + slot1 - W_local == the peer's chunks, exactly.
            # On GpSimd so it never queues behind DVE evictions.
            wp = w_pool.tile([128, 2, 2, NT, 128], bf16)
            for s in range(2):
                nc.scalar.dma_start(wp[:, s], w_pair[s])
            wsum = w_pool.tile([128, NT, 128], f32)
            wpeer = w_pool.tile([128, 2, NT, 128], bf16)
            with scope("peer"):
                for ec in range(2):
                    nc.gpsimd.tensor_add(wsum[:], wp[:, 0, ec], wp[:, 1, ec])
                    nc.gpsimd.tensor_sub(wpeer[:, ec], wsum[:], wl[:, ec])

            def sweep(ps_slice, w_ec, xt):
                for t in range(NT):
                    nc.tensor.matmul(
                        ps_slice,
                        w_ec[:, t, :],
                        xt[:, t, :],
                        start=(t == 0),
                        stop=(t == NT - 1),
                    )

            def evict(ps_slice, bias_slice, osb, mj, r0):
                nc.vector.scalar_tensor_tensor(
                    osb[:], ps_slice, SCALE, bias_slice, mult, add
                )
                n = osb.shape[1]
                for k in range(n):
                    eng = nc.scalar if (mj + k) % 2 == 0 else nc.sync
                    eng.dma_start(
                        outT[(r0 + k) * 128:(r0 + k + 1) * 128,
                             mj * 512:(mj + 1) * 512],
                        osb[:, k],
                    )

            def prefetch(i):
                if i < NMJ:
                    eng = nc.gpsimd if i % 2 == 0 else nc.scalar
                    eng.dma_start(xsb[i][:], xh[i])

            with (
                tc.tile_pool(name="osb", bufs=2) as o_pool,
                tc.tile_pool(name="ps", bufs=2, space="PSUM") as p_pool,
            ):
                # Phase A: first PRE m-tiles x local cols (AllGather shadow).
                for mj in range(PRE):
                    with scope(f"gA{mj}"):
                        ps = p_pool.tile([128, 4, 512], f32, tag="ps")
                        sweep(ps[:, 0, :], wl[:, 0], xsb[mj])
                        sweep(ps[:, 1, :], wl[:, 1], xsb[mj])
                        osb = o_pool.tile([128, 2, 512], f32, tag="osb")
                        evict(ps[:, 0:2], bias_sb[:, 0:2], osb, mj, 0)
                # Phase B: same m-tiles x peer cols; prefetch late x tiles
                # as slots free up.
                for mj in range(PRE):
                    with scope(f"gB{mj}"):
                        ps = p_pool.tile([128, 4, 512], f32, tag="ps")
                        sweep(ps[:, 0, :], wpeer[:, 0], xsb[mj])
                        sweep(ps[:, 1, :], wpeer[:, 1], xsb[mj])
                        prefetch(5 + mj)
                        osb = o_pool.tile([128, 2, 512], f32, tag="osb")
                        evict(ps[:, 0:2], bias_sb[:, 2:4], osb, mj, 2)
                # Phase C: remaining m-tiles x all four e-tiles.
                for mj in range(PRE, NMJ):
                    with scope(f"gC{mj}"):
                        ps = p_pool.tile([128, 4, 512], f32, tag="ps")
                        sweep(ps[:, 0, :], wl[:, 0], xsb[mj])
                        sweep(ps[:, 1, :], wl[:, 1], xsb[mj])
                        sweep(ps[:, 2, :], wpeer[:, 0], xsb[mj])
                        sweep(ps[:, 3, :], wpeer[:, 1], xsb[mj])
                        prefetch(mj + 5)
                        osb = o_pool.tile([128, 4, 512], f32, tag="osb")
                        evict(ps[:], bias_sb[:], osb, mj, 0)

    nc.compile()
    return nc


def _get_nc():
    if "nc" not in _CACHE:
        _CACHE["nc"] = _build()
    return _CACHE["nc"]


def _shard(x, factors, bias):
    import ml_dtypes

    bf = ml_dtypes.bfloat16
    x_flat = np.asarray(x, dtype=np.float32).reshape(B * T, DIM).astype(bf)
    factors = np.asarray(factors, dtype=np.float32).astype(bf)
    bias = np.ascontiguousarray(bias, dtype=np.float32)
    in_maps = []
    for c in range(N_CORES):
        dp, tp = c // TP, c % TP
        xc = x_flat[dp * MC:(dp + 1) * MC, :]           # [m, d]
        # -> [mj, p, t, m] with d = t*128+p, m = mj*512+m'
        xh = np.ascontiguousarray(
            xc.T.reshape(NT, 128, NMJ, 512).transpose(2, 1, 0, 3)
        )
        c0 = tp * ECO + dp * ECL
        fc = factors[:, :, c0:c0 + ECL]                 # [r, d, e]
        # -> [ec, p, rp, q, t, e]
        fhc = np.ascontiguousarray(
            fc.reshape(RANK // 2, 2, NT, 128, 2, 128).transpose(4, 3, 0, 1, 2, 5)
        )
        colmap = [tp * ECO + dp * ECL, tp * ECO + dp * ECL + 128,
                  tp * ECO + (1 - dp) * ECL, tp * ECO + (1 - dp) * ECL + 128]
        b4 = np.stack([bias[cm:cm + 128] for cm in colmap], axis=1)  # [128, 4]
        bias_melt = np.ascontiguousarray(
            np.broadcast_to(b4[:, :, None], (128, 4, 512)), dtype=np.float32
        )
        in_maps.append({"xh": xh, "fh": fhc, "bias_melt": bias_melt})
    return in_maps


def _run(in_maps, trace=False, trace_cores=None):
    from concourse.bass_utils import run_bass_kernel_spmd

    nc = _get_nc()
    return run_bass_kernel_spmd(
        nc, in_maps, list(range(N_CORES)), trace=trace, trace_cores=trace_cores
    )


def _assemble(res):
    out = np.empty((B * T, DIM), dtype=np.float32)
    for c in range(N_CORES):
        dp, tp = c // TP, c % TP
        outT = res.results[c]["outT"]
        colmap = [tp * ECO + dp * ECL, tp * ECO + dp * ECL + 128,
                  tp * ECO + (1 - dp) * ECL, tp * ECO + (1 - dp) * ECL + 128]
        for k, cm in enumerate(colmap):
            out[dp * MC:(dp + 1) * MC, cm:cm + 128] = \
                outT[k * 128:(k + 1) * 128, :].T
    return out.reshape(B, T, DIM)


def kernel(x, factors, bias):
    res = _run(_shard(x, factors, bias), trace=False)
    return _assemble(res)


# revision 10
# speedup vs baseline: 1.4265x; 1.0880x over previous
"""Trainium2 kernel for nn_BalancedHamiltonLayer.

Math: out = einsum("btd,rde->bte", x, factors)/sqrt(rank) + bias.
The einsum contracts r as a plain sum, so sum_r (x @ F_r) == x @ (sum_r F_r):
one [16384,2048] @ [2048,2048] GEMM instead of eight.

Distribution over 8 NeuronCores (single SPMD program), DP2 x TP4:
  - core c = (dp, tp) with dp = c//4, tp = c%4 owns output rows
    [dp*8192, (dp+1)*8192) and output cols [tp*512, (tp+1)*512).
  - W reduction is split between the two cores sharing a tp column group:
    core (dp, tp) reduces cols [tp*512 + dp*256, +256) from its factor
    slice (bf16 strided-pair adds on DVE: 3 instructions per 128-col
    chunk instead of a 7-add tree), then one tiny pairwise AllGather
    (1 MB, groups [[0,4],[1,5],[2,6],[3,7]]) swaps the halves.
  - SPMD slot trick: every core computes W_peer = (slot0 + slot1) -
    W_local in fp32 (exact: W_local equals one slot bitwise, so the
    subtraction recovers the other slot's bf16 values exactly). This
    keeps the program identical across cores with no per-core indexing.
  - GEMM in bf16 (fp32 PSUM): phase A = first PRE m-tiles against the
    locally reduced cols (runs in the AllGather's shadow), phase B =
    same m-tiles against peer cols, phase C = remaining m-tiles against
    all four 128-col e-tiles. 1/sqrt(8) scale + bias fused into the
    PSUM eviction (scalar_tensor_tensor on DVE).
  - Each core writes out^T [512e, 8192m] fp32; the host transposes back.

Host side shards/lays out inputs (partition-major swizzle, bf16 cast) and
reassembles the per-core outputs.
"""

import math

import numpy as np

B, T, DIM, RANK = 4, 4096, 2048, 8
N_CORES = 8
DP, TP = 2, 4
MC = (B * T) // DP             # 8192 rows per core
ECO = DIM // TP                # 512 output cols per core
ECL = ECO // DP                # 256 cols reduced locally
NT = DIM // 128                # 16 contraction tiles
NMJ = MC // 512                # 16 m-supertiles per core
PRE = 6                        # m-tiles run split (local cols first)
SCALE = 1.0 / math.sqrt(RANK)

_CACHE = {}


def _build():
    import concourse.bacc as bacc
    import concourse.mybir as mybir
    import concourse.tile as tile

    f32 = mybir.dt.float32
    bf16 = mybir.dt.bfloat16
    add = mybir.AluOpType.add
    mult = mybir.AluOpType.mult
    pair_groups = [[2 * g, 2 * g + 1] for g in range(4)]

    nc = bacc.Bacc(
        "TRN2", target_bir_lowering=False, debug=False, num_devices=N_CORES
    )
    # x^T tiles: d = t*128+p, m_global = dp*8192 + mj*512 + m
    xh = nc.dram_tensor("xh", [NMJ, 128, NT, 512], bf16, kind="ExternalInput").ap()
    # local factor cols: [ec, p, rp, q, t, e]; r = rp*2+q, d = t*128+p,
    # e_global = tp*512 + dp*256 + ec*128 + e
    fh = nc.dram_tensor(
        "fh", [2, 128, RANK // 2, 2, NT, 128], bf16, kind="ExternalInput"
    ).ap()
    # bias per (p, slot) replicated over m; slot order [L0, L1, P0, P1]
    bias_melt = nc.dram_tensor(
        "bias_melt", [128, 4, 512], f32, kind="ExternalInput"
    ).ap()
    # transposed output: rows = [L0 L1 P0 P1] col chunks, cols = m
    outT = nc.dram_tensor("outT", [ECO, MC], f32, kind="ExternalOutput").ap()

    with tile.TileContext(nc) as tc:
        with (
            tc.tile_pool(name="const", bufs=1) as const_pool,
            tc.tile_pool(name="dram", bufs=1, space="DRAM") as dram_pool,
            tc.tile_pool(name="wsb", bufs=1) as w_pool,
            tc.tile_pool(name="xa", bufs=6) as x_pool,
        ):
            scope = nc.named_scope
            xsb = [x_pool.tile([128, NT, 512], bf16, tag="x", name=f"x{i}")
                   for i in range(NMJ)]
            # x tiles serialized on gpsimd so they never starve the factor
            # loads (sync/scalar), which gate the whole pipeline.
            for mj in range(6):
                nc.gpsimd.dma_start(xsb[mj][:], xh[mj])

            wc = dram_pool.tile([128, 2, NT, 128], bf16, name="wc")
            w_pair = dram_pool.tile([2, 128, 2, NT, 128], bf16, name="w_pair")

            # Phase R: local W chunks. One 1 MB factor load per e-chunk,
            # f0 on sync and f1 on scalar in parallel at t=0; 3
            # strided-pair adds per chunk on DVE (bf16 2x mode).
            wl = w_pool.tile([128, 2, NT, 128], bf16)
            with tc.tile_pool(name="red", bufs=1) as red_pool:
                fsb = []
                for ec in range(2):
                    ft = red_pool.tile([128, RANK // 2, 2, NT, 128], bf16,
                                       tag="fr")
                    (nc.sync if ec == 0 else nc.scalar).dma_start(ft[:], fh[ec])
                    fsb.append(ft)
                bias_sb = const_pool.tile([128, 4, 512], f32)
                nc.scalar.dma_start(bias_sb[:], bias_melt[:])
                for ec in range(2):
                    with scope(f"reduce{ec}"):
                        s1 = red_pool.tile([128, RANK // 2, NT, 128], bf16,
                                           tag="s1")
                        nc.vector.tensor_add(s1[:], fsb[ec][:, :, 0],
                                             fsb[ec][:, :, 1])
                        s2 = red_pool.tile([128, 2, NT, 128], bf16, tag="s2")
                        nc.vector.tensor_add(s2[:], s1[:, 0:2], s1[:, 2:4])
                        nc.vector.tensor_add(wl[:, ec], s2[:, 0], s2[:, 1])
                        nc.sync.dma_start(wc[:, ec], wl[:, ec])

            # Pairwise AllGather of the two local chunks (1 MB in, 2 MB out).
            with scope("ag"):
                nc.gpsimd.collective_compute(
                    "AllGather", mybir.AluOpType.bypass,
                    ins=[wc.opt()],
                    outs=[w_pair.opt()],
                    replica_groups=pair_groups,
                )

            # Peer W: slot0 + slot1 - W_local == the peer's chunks, exactly
            # (W_local is bitwise one of the slots, so fp32 add/sub recovers
            # the other slot's bf16 values). Loads split per (slot, ec) so
            # wpeer[:, 0] is ready before wpeer[:, 1] is needed; the DVE ops
            # are issued after phase A to stay behind its evictions.
            wp = w_pool.tile([128, 2, 2, NT, 128], bf16)
            for ec in range(2):
                for s in range(2):
                    eng = nc.sync if s == 0 else nc.scalar
                    eng.dma_start(wp[:, s, ec], w_pair[s][:, ec])
            wsum = w_pool.tile([128, NT, 128], f32)
            wpeer = w_pool.tile([128, 2, NT, 128], bf16)

            def peer_ops():
                with scope("peer"):
                    for ec in range(2):
                        nc.vector.tensor_add(wsum[:], wp[:, 0, ec],
                                             wp[:, 1, ec])
                        nc.vector.tensor_sub(wpeer[:, ec], wsum[:], wl[:, ec])

            def sweep(ps_slice, w_ec, xt):
                for t in range(NT):
                    nc.tensor.matmul(
                        ps_slice,
                        w_ec[:, t, :],
                        xt[:, t, :],
                        start=(t == 0),
                        stop=(t == NT - 1),
                    )

            def evict(ps_slice, bias_slice, osb, mj, r0):
                nc.vector.scalar_tensor_tensor(
                    osb[:], ps_slice, SCALE, bias_slice, mult, add
                )
                n = osb.shape[1]
                for k in range(n):
                    eng = nc.scalar if (mj + k) % 2 == 0 else nc.sync
                    eng.dma_start(
                        outT[(r0 + k) * 128:(r0 + k + 1) * 128,
                             mj * 512:(mj + 1) * 512],
                        osb[:, k],
                    )

            def prefetch(i):
                if i < NMJ:
                    eng = nc.gpsimd if i % 2 == 0 else nc.scalar
                    eng.dma_start(xsb[i][:], xh[i])

            with (
                tc.tile_pool(name="osb", bufs=2) as o_pool,
                tc.tile_pool(name="ps", bufs=2, space="PSUM") as p_pool,
            ):
                # Phase A: first PRE m-tiles x local cols (AllGather shadow).
                for mj in range(PRE):
                    with scope(f"gA{mj}"):
                        ps = p_pool.tile([128, 4, 512], f32, tag="ps")
                        sweep(ps[:, 0, :], wl[:, 0], xsb[mj])
                        sweep(ps[:, 1, :], wl[:, 1], xsb[mj])
                        osb = o_pool.tile([128, 2, 512], f32, tag="osb")
                        evict(ps[:, 0:2], bias_sb[:, 0:2], osb, mj, 0)
                peer_ops()
                # Phase B: same m-tiles x peer cols; prefetch late x tiles
                # as slots free up.
                for mj in range(PRE):
                    with scope(f"gB{mj}"):
                        ps = p_pool.tile([128, 4, 512], f32, tag="ps")
                        sweep(ps[:, 0, :], wpeer[:, 0], xsb[mj])
                        sweep(ps[:, 1, :], wpeer[:, 1], xsb[mj])
                        prefetch(6 + mj)
                        osb = o_pool.tile([128, 2, 512], f32, tag="osb")
                        evict(ps[:, 0:2], bias_sb[:, 2:4], osb, mj, 2)
                # Phase C: remaining m-tiles x all four e-tiles.
                for mj in range(PRE, NMJ):
                    with scope(f"gC{mj}"):
                        ps = p_pool.tile([128, 4, 512], f32, tag="ps")
                        sweep(ps[:, 0, :], wl[:, 0], xsb[mj])
                        sweep(ps[:, 1, :], wl[:, 1], xsb[mj])
                        sweep(ps[:, 2, :], wpeer[:, 0], xsb[mj])
                        sweep(ps[:, 3, :], wpeer[:, 1], xsb[mj])
                        prefetch(mj + 6)
                        osb = o_pool.tile([128, 4, 512], f32, tag="osb")
                        evict(ps[:], bias_sb[:], osb, mj, 0)

    nc.compile()
    return nc


def _get_nc():
    if "nc" not in _CACHE:
        _CACHE["nc"] = _build()
    return _CACHE["nc"]


def _shard(x, factors, bias):
    import ml_dtypes

    bf = ml_dtypes.bfloat16
    x_flat = np.asarray(x, dtype=np.float32).reshape(B * T, DIM).astype(bf)
    factors = np.asarray(factors, dtype=np.float32).astype(bf)
    bias = np.ascontiguousarray(bias, dtype=np.float32)
    in_maps = []
    for c in range(N_CORES):
        tp, dp = c // DP, c % DP
        xc = x_flat[dp * MC:(dp + 1) * MC, :]           # [m, d]
        # -> [mj, p, t, m] with d = t*128+p, m = mj*512+m'
        xh = np.ascontiguousarray(
            xc.T.reshape(NT, 128, NMJ, 512).transpose(2, 1, 0, 3)
        )
        c0 = tp * ECO + dp * ECL
        fc = factors[:, :, c0:c0 + ECL]                 # [r, d, e]
        # -> [ec, p, rp, q, t, e]
        fhc = np.ascontiguousarray(
            fc.reshape(RANK // 2, 2, NT, 128, 2, 128).transpose(4, 3, 0, 1, 2, 5)
        )
        colmap = [tp * ECO + dp * ECL, tp * ECO + dp * ECL + 128,
                  tp * ECO + (1 - dp) * ECL, tp * ECO + (1 - dp) * ECL + 128]
        b4 = np.stack([bias[cm:cm + 128] for cm in colmap], axis=1)  # [128, 4]
        bias_melt = np.ascontiguousarray(
            np.broadcast_to(b4[:, :, None], (128, 4, 512)), dtype=np.float32
        )
        in_maps.append({"xh": xh, "fh": fhc, "bias_melt": bias_melt})
    return in_maps


def _run(in_maps, trace=False, trace_cores=None):
    from concourse.bass_utils import run_bass_kernel_spmd

    nc = _get_nc()
    return run_bass_kernel_spmd(
        nc, in_maps, list(range(N_CORES)), trace=trace, trace_cores=trace_cores
    )


def _assemble(res):
    out = np.empty((B * T, DIM), dtype=np.float32)
    for c in range(N_CORES):
        tp, dp = c // DP, c % DP
        outT = res.results[c]["outT"]
        colmap = [tp * ECO + dp * ECL, tp * ECO + dp * ECL + 128,
                  tp * ECO + (1 - dp) * ECL, tp * ECO + (1 - dp) * ECL + 128]
        for k, cm in enumerate(colmap):
            out[dp * MC:(dp + 1) * MC, cm:cm + 128] = \
                outT[k * 128:(k + 1) * 128, :].T
    return out.reshape(B, T, DIM)


def kernel(x, factors, bias):
    res = _run(_shard(x, factors, bias), trace=False)
    return _assemble(res)


# revision 13
# speedup vs baseline: 1.4314x; 1.0035x over previous
"""Trainium2 kernel for nn_BalancedHamiltonLayer.

Math: out = einsum("btd,rde->bte", x, factors)/sqrt(rank) + bias.
The einsum contracts r as a plain sum, so sum_r (x @ F_r) == x @ (sum_r F_r):
one [16384,2048] @ [2048,2048] GEMM instead of eight.

Distribution over 8 NeuronCores (single SPMD program), DP2 x TP4:
  - core c = (dp, tp) with dp = c//4, tp = c%4 owns output rows
    [dp*8192, (dp+1)*8192) and output cols [tp*512, (tp+1)*512).
  - W reduction is split between the two cores sharing a tp column group:
    core (dp, tp) reduces cols [tp*512 + dp*256, +256) from its factor
    slice (bf16 strided-pair adds on DVE: 3 instructions per 128-col
    chunk instead of a 7-add tree), then one tiny pairwise AllGather
    (1 MB, groups [[0,4],[1,5],[2,6],[3,7]]) swaps the halves.
  - SPMD slot trick: every core computes W_peer = (slot0 + slot1) -
    W_local in fp32 (exact: W_local equals one slot bitwise, so the
    subtraction recovers the other slot's bf16 values exactly). This
    keeps the program identical across cores with no per-core indexing.
  - GEMM in bf16 (fp32 PSUM): phase A = first PRE m-tiles against the
    locally reduced cols (runs in the AllGather's shadow), phase B =
    same m-tiles against peer cols, phase C = remaining m-tiles against
    all four 128-col e-tiles. 1/sqrt(8) scale + bias fused into the
    PSUM eviction (scalar_tensor_tensor on DVE).
  - Each core writes out^T [512e, 8192m] fp32; the host transposes back.

Host side shards/lays out inputs (partition-major swizzle, bf16 cast) and
reassembles the per-core outputs.
"""

import math

import numpy as np

B, T, DIM, RANK = 4, 4096, 2048, 8
N_CORES = 8
DP, TP = 2, 4
MC = (B * T) // DP             # 8192 rows per core
ECO = DIM // TP                # 512 output cols per core
ECL = ECO // DP                # 256 cols reduced locally
NT = DIM // 128                # 16 contraction tiles
NMJ = MC // 512                # 16 m-supertiles per core
PRE = 6                        # m-tiles run split (local cols first)
SCALE = 1.0 / math.sqrt(RANK)

_CACHE = {}


def _build():
    import concourse.bacc as bacc
    import concourse.mybir as mybir
    import concourse.tile as tile

    f32 = mybir.dt.float32
    bf16 = mybir.dt.bfloat16
    add = mybir.AluOpType.add
    mult = mybir.AluOpType.mult
    pair_groups = [[2 * g, 2 * g + 1] for g in range(4)]

    nc = bacc.Bacc(
        "TRN2", target_bir_lowering=False, debug=False, num_devices=N_CORES
    )
    # x^T tiles: d = t*128+p, m_global = dp*8192 + mj*512 + m
    xh = nc.dram_tensor("xh", [NMJ, 128, NT, 512], bf16, kind="ExternalInput").ap()
    # local factor cols: [ec, p, rp, q, t, e]; r = rp*2+q, d = t*128+p,
    # e_global = tp*512 + dp*256 + ec*128 + e
    fh = nc.dram_tensor(
        "fh", [2, 128, RANK // 2, 2, NT, 128], bf16, kind="ExternalInput"
    ).ap()
    # bias per (p, slot) replicated over m; slot order [L0, L1, P0, P1]
    bias_melt = nc.dram_tensor(
        "bias_melt", [128, 4, 512], f32, kind="ExternalInput"
    ).ap()
    # transposed output: rows = [L0 L1 P0 P1] col chunks, cols = m
    outT = nc.dram_tensor("outT", [ECO, MC], f32, kind="ExternalOutput").ap()

    with tile.TileContext(nc) as tc:
        with (
            tc.tile_pool(name="const", bufs=1) as const_pool,
            tc.tile_pool(name="dram", bufs=1, space="DRAM") as dram_pool,
            tc.tile_pool(name="wsb", bufs=1) as w_pool,
            tc.tile_pool(name="xa", bufs=6) as x_pool,
        ):
            scope = nc.named_scope
            xsb = [x_pool.tile([128, NT, 512], bf16, tag="x", name=f"x{i}")
                   for i in range(NMJ)]
            # Factor loads + bias enqueue FIRST: the physical DMA queues
            # drain roughly in enqueue order, so anything issued before
            # them adds straight latency to the reduction -> AllGather ->
            # peer-W critical path.
            ftiles = []
            with tc.tile_pool(name="red", bufs=1) as red_pool:
                for ec in range(2):
                    ft = red_pool.tile([128, RANK // 2, 2, NT, 128], bf16,
                                       tag="fr", name=f"f{ec}")
                    (nc.sync if ec == 0 else nc.scalar).dma_start(ft[:], fh[ec])
                    ftiles.append(ft)
                bias_sb = const_pool.tile([128, 4, 512], f32)
                nc.scalar.dma_start(bias_sb[:], bias_melt[:])

                # Early x tiles on gpsimd; x3..x5 staggered so the wc
                # stores and AllGather payloads find queue slack.
                nc.gpsimd.dma_start(xsb[0][:], xh[0])
                nc.gpsimd.dma_start(xsb[1][:], xh[1])
                nc.gpsimd.dma_start(xsb[2][:], xh[2])
                for mj, ms in ((3, 0.015), (4, 0.024), (5, 0.033)):
                    with tc.tile_wait_until(ms):
                        nc.gpsimd.dma_start(xsb[mj][:], xh[mj])

                wcs, w_pairs = [], []
                for ec in range(2):
                    wcs.append(dram_pool.tile([128, NT, 128], bf16,
                                              name=f"wc{ec}"))
                    w_pairs.append(dram_pool.tile([2, 128, NT, 128], bf16,
                                                  name=f"w_pair{ec}"))

                # Phase R: local W chunks; 3 strided-pair adds per chunk on
                # DVE (bf16 2x mode); each chunk's pairwise AllGather fires
                # as soon as that chunk's store lands.
                wl = w_pool.tile([128, 2, NT, 128], bf16)
                for ec in range(2):
                    with scope(f"reduce{ec}"):
                        s1 = red_pool.tile([128, RANK // 2, NT, 128], bf16,
                                           tag="s1")
                        nc.vector.tensor_add(s1[:], ftiles[ec][:, :, 0],
                                             ftiles[ec][:, :, 1])
                        s2 = red_pool.tile([128, 2, NT, 128], bf16, tag="s2")
                        nc.vector.tensor_add(s2[:], s1[:, 0:2], s1[:, 2:4])
                        nc.vector.tensor_add(wl[:, ec], s2[:, 0], s2[:, 1])
                        nc.sync.dma_start(wcs[ec][:], wl[:, ec])
                    with scope(f"ag{ec}"):
                        nc.gpsimd.collective_compute(
                            "AllGather", mybir.AluOpType.bypass,
                            ins=[wcs[ec].opt()],
                            outs=[w_pairs[ec].opt()],
                            replica_groups=pair_groups,
                        )

            # Peer W inputs: all on the scalar ring so the (late-firing)
            # triggers never block evict stores, which all go on sync.
            wp = [w_pool.tile([128, 2, NT, 128], bf16, name=f"wp{ec}")
                  for ec in range(2)]
            for ec in range(2):
                for s in range(2):
                    nc.scalar.dma_start(wp[ec][:, s], w_pairs[ec][s])
            wsum = w_pool.tile([128, NT, 128], f32)
            wpeer = w_pool.tile([128, 2, NT, 128], bf16)

            def peer_ops(ec):
                # slot0 + slot1 - W_local == the peer chunk, exactly
                # (W_local is bitwise one of the slots).
                with scope(f"peer{ec}"):
                    nc.vector.tensor_add(wsum[:], wp[ec][:, 0], wp[ec][:, 1])
                    nc.vector.tensor_sub(wpeer[:, ec], wsum[:], wl[:, ec])

            def sweep(ps_slice, w_ec, xt):
                for t in range(NT):
                    nc.tensor.matmul(
                        ps_slice,
                        w_ec[:, t, :],
                        xt[:, t, :],
                        start=(t == 0),
                        stop=(t == NT - 1),
                    )

            def evict(ps_slice, bias_slice, osb, mj, r0):
                nc.vector.scalar_tensor_tensor(
                    osb[:], ps_slice, SCALE, bias_slice, mult, add
                )
                for k in range(osb.shape[1]):
                    nc.sync.dma_start(
                        outT[(r0 + k) * 128:(r0 + k + 1) * 128,
                             mj * 512:(mj + 1) * 512],
                        osb[:, k],
                    )

            def prefetch(i):
                if i < NMJ:
                    nc.gpsimd.dma_start(xsb[i][:], xh[i])

            with (
                tc.tile_pool(name="osb", bufs=3) as o_pool,
                tc.tile_pool(name="ps", bufs=2, space="PSUM") as p_pool,
            ):
                # Phase A: first PRE m-tiles x local cols (AllGather shadow).
                # peer_ops(0) is slotted into the DVE stream near the end of
                # A: late enough not to head-of-line block A's evictions
                # behind the AllGather, early enough to unblock phase B0.
                for mj in range(PRE):
                    with scope(f"gA{mj}"):
                        ps = p_pool.tile([128, 4, 512], f32, tag="ps")
                        sweep(ps[:, 0, :], wl[:, 0], xsb[mj])
                        sweep(ps[:, 1, :], wl[:, 1], xsb[mj])
                        osb = o_pool.tile([128, 2, 512], f32, tag="osb")
                        evict(ps[:, 0:2], bias_sb[:, 0:2], osb, mj, 0)
                        if mj == PRE - 2:
                            peer_ops(0)
                # Phase B0: same m-tiles x first peer chunk (single-sweep
                # units; the L/P column blocks are independent outputs, so
                # the two peer chunks can land as separate passes).
                for mj in range(PRE):
                    with scope(f"gB0_{mj}"):
                        ps = p_pool.tile([128, 4, 512], f32, tag="ps")
                        sweep(ps[:, 0, :], wpeer[:, 0], xsb[mj])
                        osb = o_pool.tile([128, 1, 512], f32, tag="osb")
                        evict(ps[:, 0:1], bias_sb[:, 2:3], osb, mj, 2)
                        if mj == 2:
                            peer_ops(1)
                # Phase B1: same m-tiles x second peer chunk.
                for mj in range(PRE):
                    with scope(f"gB1_{mj}"):
                        ps = p_pool.tile([128, 4, 512], f32, tag="ps")
                        sweep(ps[:, 0, :], wpeer[:, 1], xsb[mj])
                        prefetch(PRE + mj)
                        osb = o_pool.tile([128, 1, 512], f32, tag="osb")
                        evict(ps[:, 0:1], bias_sb[:, 3:4], osb, mj, 3)
                # Phase C: remaining m-tiles x all four e-tiles.
                for mj in range(PRE, NMJ):
                    with scope(f"gC{mj}"):
                        ps = p_pool.tile([128, 4, 512], f32, tag="ps")
                        sweep(ps[:, 0, :], wl[:, 0], xsb[mj])
                        sweep(ps[:, 1, :], wl[:, 1], xsb[mj])
                        sweep(ps[:, 2, :], wpeer[:, 0], xsb[mj])
                        sweep(ps[:, 3, :], wpeer[:, 1], xsb[mj])
                        prefetch(mj + PRE)
                        osb = o_pool.tile([128, 4, 512], f32, tag="osb")
                        evict(ps[:], bias_sb[:], osb, mj, 0)

    nc.compile()
    return nc


def _get_nc():
    if "nc" not in _CACHE:
        _CACHE["nc"] = _build()
    return _CACHE["nc"]


def _shard(x, factors, bias):
    import ml_dtypes

    bf = ml_dtypes.bfloat16
    x_flat = np.asarray(x, dtype=np.float32).reshape(B * T, DIM).astype(bf)
    factors = np.asarray(factors, dtype=np.float32).astype(bf)
    bias = np.ascontiguousarray(bias, dtype=np.float32)
    in_maps = []
    for c in range(N_CORES):
        tp, dp = c // DP, c % DP
        xc = x_flat[dp * MC:(dp + 1) * MC, :]           # [m, d]
        # -> [mj, p, t, m] with d = t*128+p, m = mj*512+m'
        xh = np.ascontiguousarray(
            xc.T.reshape(NT, 128, NMJ, 512).transpose(2, 1, 0, 3)
        )
        c0 = tp * ECO + dp * ECL
        fc = factors[:, :, c0:c0 + ECL]                 # [r, d, e]
        # -> [ec, p, rp, q, t, e]
        fhc = np.ascontiguousarray(
            fc.reshape(RANK // 2, 2, NT, 128, 2, 128).transpose(4, 3, 0, 1, 2, 5)
        )
        colmap = [tp * ECO + dp * ECL, tp * ECO + dp * ECL + 128,
                  tp * ECO + (1 - dp) * ECL, tp * ECO + (1 - dp) * ECL + 128]
        b4 = np.stack([bias[cm:cm + 128] for cm in colmap], axis=1)  # [128, 4]
        bias_melt = np.ascontiguousarray(
            np.broadcast_to(b4[:, :, None], (128, 4, 512)), dtype=np.float32
        )
        in_maps.append({"xh": xh, "fh": fhc, "bias_melt": bias_melt})
    return in_maps


def _run(in_maps, trace=False, trace_cores=None):
    from concourse.bass_utils import run_bass_kernel_spmd

    nc = _get_nc()
    return run_bass_kernel_spmd(
        nc, in_maps, list(range(N_CORES)), trace=trace, trace_cores=trace_cores
    )


def _assemble(res):
    out = np.empty((B * T, DIM), dtype=np.float32)
    for c in range(N_CORES):
        tp, dp = c // DP, c % DP
        outT = res.results[c]["outT"]
        colmap = [tp * ECO + dp * ECL, tp * ECO + dp * ECL + 128,
                  tp * ECO + (1 - dp) * ECL, tp * ECO + (1 - dp) * ECL + 128]
        for k, cm in enumerate(colmap):
            out[dp * MC:(dp + 1) * MC, cm:cm + 128] = \
                outT[k * 128:(k + 1) * 128, :].T
    return out.reshape(B, T, DIM)


def kernel(x, factors, bias):
    res = _run(_shard(x, factors, bias), trace=False)
    return _assemble(res)
